# revision 5
# baseline (speedup 1.0000x reference)
"""AttentionWithContext pooling kernel for Trainium2 (8 NeuronCores, data-parallel).

Computation (per batch row, matching the reference):
    uit = tanh(x @ W + b)          # (T, F)
    ait = uit @ u                  # (T,)
    a   = exp(ait); a /= (sum(a) + 1e-7)
    out = x.T @ a                  # (F,)

Sharding: pure data parallel over the batch dim. B=64 batches -> 8 per core.
All on-chip compute is bf16 (fp32 PSUM accumulation); end-to-end error vs the
fp32 reference is ~2e-3 relative to output absmax.

Per-core dataflow (NB=8 local batches, T=2048, F=256), quarter-grained pipeline:
  - SWDGE cast-load x[b] quarter (512 rows) HBM f32 -> SBUF bf16, natural layout
  - DMA-xbar transpose per quarter -> xT[f%128, c, f//128, t%128] (issue engine
    alternates sync/scalar to use both HWDGE rings)
  - scores:  PE  sc = W[:,gh].T @ xT   (K=F contraction, fp32 PSUM)
  - tanh:    ACT (+ per-partition bias), writes bf16 SBUF
  - u-dot:   PE  ait[1, 512] += u[gh].T @ tanh[gh]
  - exp:     ACT Exp with fused row-sum (accum_out); row -> ea_pre via gpsimd DMA
  - eaT:     xbar transpose of ea_pre [16, T] -> [t%128, c, row] per batch
  - weighted sum: PE out[1, F] += eaT[:,c,b].T @ x[b][:,c,:], scale by 1/(sum+eps)
"""

import sys

for _p in ("/opt/trn_rl_repo",):
    if _p not in sys.path:
        sys.path.append(_p)

import numpy as np

import concourse.bass as bass  # noqa: F401
import concourse.tile as tile
from concourse import bacc, mybir
from concourse.bass_utils import run_bass_kernel_spmd

B, T, F = 64, 2048, 256
NC = 8                 # cores
NB = B // NC           # batches per core
TC = T // 128          # 16 T-chunks of 128
QT = 4                 # quarters per batch (T-chunks of 512)
EPS = 1e-7

BF16 = mybir.dt.bfloat16
F32 = mybir.dt.float32


def _build_tile_kernel(nc):
    x = nc.dram_tensor("x", [NB, T, F], F32, kind="ExternalInput")
    w = nc.dram_tensor("w", [F, F], F32, kind="ExternalInput")
    bb = nc.dram_tensor("b", [F], F32, kind="ExternalInput")
    u = nc.dram_tensor("u", [F], F32, kind="ExternalInput")
    y = nc.dram_tensor("y", [NB, F], F32, kind="ExternalOutput")
    ea_dram = nc.dram_tensor("ea_scratch", [16, T], BF16)  # internal DRAM bounce

    x_r = x.ap().rearrange("b (c p) f -> b p c f", p=128)  # [NB, 128, TC, F]

    with tile.TileContext(nc) as tc:
        with (
            tc.tile_pool(name="const", bufs=1) as const,
            tc.tile_pool(name="xpool", bufs=NB) as xpool,
            tc.tile_pool(name="xtpool", bufs=2) as xtpool,
            tc.tile_pool(name="thpool", bufs=4) as thpool,
            tc.tile_pool(name="rowpool", bufs=4) as rowpool,
            tc.tile_pool(name="eatpool", bufs=2) as eatpool,
            tc.tile_pool(name="scps", bufs=3, space="PSUM") as scps,
            tc.tile_pool(name="aitps", bufs=1, space="PSUM") as aitps,
            tc.tile_pool(name="ops", bufs=1, space="PSUM") as ops,
        ):
            # ---- constants (replicated params) ----
            w_sb = const.tile([128, 2, F], BF16)          # w_sb[p, k, g] = W[k*128+p, g]
            nc.gpsimd.dma_start(out=w_sb, in_=w.ap().rearrange("(k p) g -> p k g", p=128))
            u_sb = const.tile([128, 2], BF16)             # u_sb[p, k] = u[k*128+p]
            nc.gpsimd.dma_start(out=u_sb, in_=u.ap().rearrange("(k p) -> p k", p=128))
            b_sb = const.tile([128, 2], F32)              # b_sb[p, g] = b[g*128+p]
            nc.gpsimd.dma_start(out=b_sb, in_=bb.ap().rearrange("(g p) -> p g", p=128))
            eps_sb = const.tile([1, 1], F32)
            nc.vector.memset(eps_sb, EPS)

            xb_tiles = []
            hw_eng = [nc.sync, nc.scalar]                 # two HWDGE rings
            for b in range(NB):
                xb = xpool.tile([128, TC, F], BF16, tag="xb", name=f"xb{b}")
                xb_tiles.append(xb)
                xT = xtpool.tile([128, TC, 2, 128], BF16, tag="xT", name=f"xT{b}")
                th = [thpool.tile([128, T], BF16, tag="th", name=f"th{b}_{i}")
                      for i in range(2)]
                ait = aitps.tile([1, T], F32, tag="ait", name=f"ait{b}")

                for q in range(QT):
                    cs = slice(q * 4, (q + 1) * 4)
                    # load one quarter (512 T rows): f32 HBM -> bf16 SBUF
                    nc.gpsimd.dma_start(out=xb[:, cs, :], in_=x_r[b, :, cs, :])
                    # transpose the quarter (alternate HWDGE ring)
                    hw_eng[(b * QT + q) % 2].dma_start(
                        out=xT[:, cs, :, :], in_=xb[:, cs, :], transpose=True
                    )
                    # scores + tanh + u-dot for this quarter
                    ts_ = slice(q * 512, (q + 1) * 512)
                    for gh in range(2):
                        sc = scps.tile([128, 512], F32, tag="sc")
                        for k in range(2):
                            nc.tensor.matmul(
                                sc,
                                lhsT=w_sb[:, k, gh * 128 : (gh + 1) * 128],
                                rhs=xT[:, cs, k, :],
                                start=(k == 0),
                                stop=(k == 1),
                            )
                        nc.scalar.activation(
                            out=th[gh][:, ts_],
                            in_=sc,
                            func=mybir.ActivationFunctionType.Tanh,
                            bias=b_sb[:, gh : gh + 1],
                        )
                    for gh in range(2):
                        nc.tensor.matmul(
                            ait[:, ts_],
                            lhsT=u_sb[:, gh : gh + 1],
                            rhs=th[gh][:, ts_],
                            start=(gh == 0),
                            stop=(gh == 1),
                        )

                # ---- softmax pieces ----
                ea_row = rowpool.tile([1, T], BF16, tag="ea")
                sums = rowpool.tile([1, 1], F32, tag="sums")
                nc.scalar.activation(
                    out=ea_row, in_=ait,
                    func=mybir.ActivationFunctionType.Exp, accum_out=sums,
                )
                # bounce through DRAM: avoids SBUF->SBUF DMA concurrent with
                # xbar transposes (HW hazard)
                nc.gpsimd.dma_start(out=ea_dram.ap()[b : b + 1, :], in_=ea_row)

                rsum = rowpool.tile([1, 1], F32, tag="rsum")
                nc.vector.tensor_add(rsum, sums, eps_sb)
                rinv = rowpool.tile([1, 1], F32, tag="rinv")
                nc.vector.reciprocal(rinv, rsum)

                # ---- eaT + weighted sum for this batch ----
                eaT = eatpool.tile([128, TC, 16], BF16, tag="eaT", name=f"eaT{b}")
                hw_eng[b % 2].dma_start(out=eaT, in_=ea_dram.ap(), transpose=True)

                o_ps = ops.tile([1, F], F32, tag="o")
                for c in range(TC):
                    nc.tensor.matmul(
                        o_ps,
                        lhsT=eaT[:, c, b : b + 1],
                        rhs=xb[:, c, :],
                        start=(c == 0),
                        stop=(c == TC - 1),
                    )
                o_row = rowpool.tile([1, F], F32, tag="orow")
                nc.vector.tensor_scalar_mul(o_row, o_ps, rinv)
                nc.gpsimd.dma_start(out=y.ap()[b : b + 1, :], in_=o_row)

    nc.compile()
    return nc


_NC_CACHE = None


def _get_nc():
    global _NC_CACHE
    if _NC_CACHE is None:
        nc = bacc.Bacc("TRN2", target_bir_lowering=False, debug=False)
        _NC_CACHE = _build_tile_kernel(nc)
    return _NC_CACHE


def _in_maps(x, W, b, u):
    x = np.ascontiguousarray(np.asarray(x, dtype=np.float32))
    W = np.ascontiguousarray(np.asarray(W, dtype=np.float32))
    b = np.ascontiguousarray(np.asarray(b, dtype=np.float32))
    u = np.ascontiguousarray(np.asarray(u, dtype=np.float32))
    return [
        {"x": x[i * NB : (i + 1) * NB], "w": W, "b": b, "u": u} for i in range(NC)
    ]


def kernel(x, W, b, u, _trace=False):
    nc = _get_nc()
    res = run_bass_kernel_spmd(nc, _in_maps(x, W, b, u), core_ids=list(range(NC)),
                               trace=_trace)
    out = np.concatenate([np.asarray(r["y"]) for r in res.results], axis=0)
    if _trace:
        return out, res
    return out
